# revision 33
# baseline (speedup 1.0000x reference)
"""Conv2d(128->256, 3x3, pad 1) with LoRA (rank 8) — Trainium2 Bass kernel.

Strategy:
  - Data-parallel over batch: 16 images -> 2 per core x 8 cores. Conv weights
    and LoRA A/B replicated.
  - LoRA folds into the conv weight on device (conv is linear in weights):
        W_eff = W + (alpha/rank) * (B @ A).reshape(C_OUT, C_IN, 3, 3)
    via 9 tiny PE matmuls (K=8) + fused DVE scalar_tensor_tensor adds.
  - The 3x3 conv itself = 9 shifted matmuls accumulating in PSUM:
        out[co, pix] += W_eff[co, :, kh, kw]^T @ x_shift[ci, pix]
    with K = C_IN = 128 (partition dim), M = 128 (co block), N = 512
    (8 image rows x 64 cols) in bf16 — 1 col/cycle = full PE rate; the
    288-matmul stream is the bf16 roofline (~62 us warm).
  - All tensor I/O is bf16 (host does the identical RNE rounding the DVE
    used to do; output is written bf16 and upcast on host). Halves DMA
    traffic, removes every DVE cast.
  - Head: three f32 warm-up matmuls release the PE clock gate (HAM) before
    the conv; A/B land first (one bundled DMA) -> LoRA matmuls; wt arrives
    in five fold-order pieces spread over all three DMA queues (each HW
    queue only sustains ~55-90 GB/s early and completion sems lag the data
    by ~1.5-2.5us); the conv is emitted in k-minor 4-row-group waves so
    the in-order PE queue chases the weff folds without stalling.
  - x image 1 + bias + part of wt stream via the gpsimd SWDGE queue,
    keeping both HWDGE queues free for x0/wt in and output tiles out.
  - Output tiles drain as row-group pairs (one DMA per pair); the final
    wave runs k-major with per-tile DMAs so only 128KB drains after the
    last matmul. The end-of-kernel semaphore-reset parade (~9us, fixed
    framework epilogue) plus ~6us of engine bring-up are measurement
    constants this kernel cannot remove.
"""

import numpy as np
import ml_dtypes

import concourse.bass as bass
import concourse.tile as tile
from concourse.tile import add_dep_helper
from concourse import bacc, mybir
from concourse.bass_utils import run_bass_kernel_spmd

N_CORES = 8
B, C_IN, H, W_DIM = 16, 128, 64, 64
C_OUT = 256
RANK = 8
SCALING = 2.0  # alpha/rank = 16/8
HP, WP = H + 2, W_DIM + 2  # zero-padded image dims
B_LOC = B // N_CORES  # images per core
NPIX = H * W_DIM  # 4096
ROWS_PER_TILE = 8  # output rows per matmul group -> N = 8*64 = 512
N_RG = H // ROWS_PER_TILE  # 8 row groups

F32 = mybir.dt.float32
BF16 = mybir.dt.bfloat16
IDENT = mybir.ActivationFunctionType.Identity
BF16_NP = ml_dtypes.bfloat16


def _build_nc():
    nc = bacc.Bacc(
        "TRN2",
        target_bir_lowering=False,
        debug=False,
        num_devices=N_CORES,
    )

    xp = nc.dram_tensor("xp", [B_LOC, C_IN, HP * WP], BF16, kind="ExternalInput").ap()
    wt = nc.dram_tensor("wt", [C_IN, 9 * C_OUT], BF16, kind="ExternalInput").ap()
    # at, bt and 256 zero columns bundled: [8, 9*128 | 256 | 256] -> one DMA.
    # The zero tail lets warm-filler matmuls use a 512-wide moving operand.
    ab = nc.dram_tensor(
        "ab", [RANK, 9 * C_IN + 2 * C_OUT], BF16, kind="ExternalInput"
    ).ap()
    bv = nc.dram_tensor("bv", [128, 2], F32, kind="ExternalInput").ap()
    out = nc.dram_tensor("out", [B_LOC, C_OUT, NPIX], BF16, kind="ExternalOutput").ap()

    with tile.TileContext(nc) as tc:
        with (
            tc.tile_pool(name="persist", bufs=1) as persist,
            tc.tile_pool(name="outp", bufs=6) as outp,
            tc.tile_pool(name="psum", bufs=8, space="PSUM") as psum,
        ):
            # --- persistent SBUF tiles (all bf16 straight off DMA) ----------
            x_sb = [
                persist.tile([C_IN, HP * WP], BF16, name=f"x_sb{i}")
                for i in range(B_LOC)
            ]
            wt_sb = persist.tile([C_IN, 9 * C_OUT], BF16, name="wt_sb")
            weff = persist.tile([C_IN, 9 * C_OUT], BF16, name="weff")
            ab_sb = persist.tile([RANK, 9 * C_IN + 2 * C_OUT], BF16, name="ab_sb")
            b_sb = persist.tile([128, 2], F32, name="b_sb")
            warm_sb = persist.tile([128, 512], F32, name="warm_sb")

            # --- input DMAs ------------------------------------------------
            # Queue FIFO order = priority order; each DMA_DIRECT2D costs
            # ~0.65us of issue time on its queue engine and completion sems
            # lag the data by ~1.5-2us (HBM write receipt). Critical path to
            # the first conv matmul: ab -> LoRA MMs -> (with wt q0) weff
            # fold 0; x0 rows chase the first wave's row-groups.
            qs = [nc.sync, nc.scalar]
            # Measured: each queue sustains only ~65-90 GB/s early and
            # completion sems lag data by ~1.5-2.5us, so the critical DMAs
            # sit at the FRONT of their queues and wt arrives in five
            # 512-col pieces (one per weff fold, in fold order) spread over
            # all three queues so the folds pace ~evenly.
            #   sync:   ab (gates LoRA), wt p0, wt p1
            #   scalar: x0 wave A rows (gates conv rg0-3), wt p2, x0 wave B
            #   gpsimd (SWDGE): wt p3, wt p4, bias, x1
            # p0 ahead of ab on sync: fold A (gated by p0's completion sem)
            # is on the conv critical path, while the LoRA matmuls (gated
            # by ab) are PE-queue-bound behind the f32 warm block anyway.
            nc.sync.dma_start(wt_sb[:, 0:512], wt[:, 0:512])
            nc.sync.dma_start(ab_sb[:], ab)
            xa1 = 18 * WP  # rows 0..17: conv rg0-1 (+ rg2's upper rows)
            xa = 34 * WP  # rows 0..33 cover conv wave A (rg0-3)
            nc.scalar.dma_start(x_sb[0][:, :xa1], xp[0, :, :xa1])
            nc.scalar.dma_start(x_sb[0][:, xa1:xa], xp[0, :, xa1:xa])
            nc.scalar.dma_start(wt_sb[:, 1024:1536], wt[:, 1024:1536])
            nc.scalar.dma_start(x_sb[0][:, xa:], xp[0, :, xa:])
            # The warm tile memset runs on gpsimd ahead of its DMA issues:
            # gpsimd's queue is ready ~1.3us before the DVE's, so the f32
            # warm-up matmuls (gated only by this memset) start earlier.
            nc.gpsimd.memset(warm_sb[:], 0.0)
            # wt p1 rides the otherwise-idle SWDGE queue: on sync (3rd slot
            # behind ab+p0 at ~55 GB/s early) its completion lagged to
            # ~+9us and stalled the fold chain at fold B.
            nc.gpsimd.dma_start(wt_sb[:, 512:1024], wt[:, 512:1024])
            nc.gpsimd.dma_start(wt_sb[:, 1536:2048], wt[:, 1536:2048])
            nc.gpsimd.dma_start(wt_sb[:, 2048:], wt[:, 2048:])
            nc.gpsimd.dma_start(b_sb[:], bv)
            nc.gpsimd.dma_start(x_sb[1][:], xp[1, :, :])

            # --- PE warm-up ------------------------------------------------
            # The HAM clock gate holds the PE at 1.2 GHz until ~3.4us of
            # sustained busy. Three f32 N=512 matmuls (4 cycles/row, dense
            # array activity, no DMA deps) reliably release it — bf16
            # filler streams and 2x f32 measurably do NOT.
            lps = [
                psum.tile([128, 512], F32, tag="lps", bufs=3, name=f"lps{j}")
                for j in range(5)
            ]
            for _ in range(3):
                nc.tensor.matmul(
                    lps[0][:], warm_sb[:, :128], warm_sb[:], start=True, stop=True
                )

            # --- fold LoRA into the conv weight ----------------------------
            # lps[j][:, (k%2)*256:...] = (A_k)^T @ B^T  for k = 2j, 2j+1
            # weff[:, k*256+co] = wt[:, k*256+co] + 2 * lps[...]
            # The K=8 matmuls also extend the PE warm-up. All 9 MUST be
            # emitted before the folds: Tile dependency tracking is
            # backward-looking, so a fold emitted before its lps writer
            # would silently read a stale bank (k6..k8 wait for fold A to
            # free an lps bank — they run inside the fold-B DMA window).
            for k in range(9):
                nc.tensor.matmul(
                    lps[k // 2][:, (k % 2) * 256 : (k % 2) * 256 + 256],
                    ab_sb[:, k * 128 : (k + 1) * 128],
                    ab_sb[:, 9 * C_IN : 9 * C_IN + 256],
                    start=True,
                    stop=True,
                )


            # DVE stream is FIFO and the scheduler's DMA-latency model is
            # optimistic: chain the weff folds in k order so conv wave 0
            # starts progressively off fold #0.
            def chain(inst, prev, why):
                if prev is not None:
                    add_dep_helper(inst.ins, prev.ins, sync=False, reason=why)
                return inst

            link = None
            for j in range(5):
                w = 512 if j < 4 else 256
                link = chain(
                    nc.vector.scalar_tensor_tensor(
                        weff[:, j * 512 : j * 512 + w],
                        lps[j][:, :w],
                        SCALING,
                        wt_sb[:, j * 512 : j * 512 + w],
                        op0=mybir.AluOpType.mult,
                        op1=mybir.AluOpType.add,
                    ),
                    link,
                    "weff fold k order",
                )

            # --- the conv: 9 accumulating shift-matmuls per output tile ----
            # Emitted k-minor in 4-row-group waves: the in-order PE queue
            # then needs weff fold j only ~8 matmuls after fold j-1, so it
            # chases the DVE chain without stalling, and each weight is
            # loaded once per wave instead of once per tile.
            first_wave = True
            for img in range(B_LOC):
                x_r = x_sb[img][:].rearrange("p (h w) -> p h w", w=WP)
                for cb in range(2):
                    for wv in range(2):
                        rgs = [wv * 4 + j for j in range(4)]
                        last_wave = img == B_LOC - 1 and cb == 1 and wv == 1
                        # units: (rg, row offset within rg, n rows). The
                        # LAST row group is split into two 4-row half
                        # tiles in separate PSUM banks: their bias-adds
                        # then run on ACT and DVE in PARALLEL and two
                        # 64KB DMAs drain on both queues, shortening the
                        # final post-matmul drain.
                        if not last_wave:
                            units = [(rg, 0, 8) for rg in rgs]
                        else:
                            units = [(4, 0, 8), (5, 0, 8), (6, 0, 8),
                                     (7, 0, 4), (7, 4, 4)]
                        ps = {
                            u: psum.tile(
                                [128, u[2] * 64], F32, tag="ps", bufs=5,
                                name=f"ps{img}_{cb}_{u[0]}_{u[1]}",
                            )
                            for u in units
                        }
                        # k-minor: the PE chases the weff folds (needed for
                        # the first waves). The LAST wave is k-major so its
                        # tiles COMPLETE staggered and their drains overlap
                        # the matmuls instead of piling up at the end.
                        kloop = (
                            [(k, u) for k in range(9) for u in units]
                            if not last_wave
                            else [(k, u) for u in units for k in range(9)]
                        )
                        for k, u in kloop:
                            rg, roff, nrows = u
                            dh, dw = k // 3 - 1, k % 3 - 1
                            lhsT = weff[
                                :, k * 256 + cb * 128 : k * 256 + cb * 128 + 128
                            ]
                            h0 = rg * ROWS_PER_TILE + roff
                            rhs = x_r[
                                :,
                                h0 + 1 + dh : h0 + 1 + dh + nrows,
                                1 + dw : 65 + dw,
                            ]
                            nc.tensor.matmul(
                                ps[u][:],
                                lhsT,
                                rhs,
                                start=(k == 0),
                                stop=(k == 8),
                            )

                        # drain: PSUM -> bf16 SBUF (+bias) per unit,
                        # alternating ACT/DVE; one out-DMA per rg PAIR
                        # (fewer DMA instructions -> less issue time on the
                        # queues). The FINAL wave uses one DMA per unit on
                        # alternating queues so only the last 64KB half
                        # tile drains after the final matmul.
                        if not last_wave:
                            prs = [(units[0], units[1]), (units[2], units[3])]
                        else:
                            prs = [(u,) for u in units]
                        for pi, pair in enumerate(prs):
                            wid = sum(64 * u[2] for u in pair)
                            o = outp.tile(
                                [128, wid], BF16, tag="o",
                                name=f"o{img}_{cb}_{pair[0][0]}_{pair[0][1]}",
                            )
                            col0 = pair[0][0] * 512 + pair[0][1] * 64
                            oc = 0
                            for h, u in enumerate(pair):
                                uw = 64 * u[2]
                                ti = (img * 2 + cb) * N_RG + u[0] + u[1] // 4
                                if (ti + (h if not last_wave else 0)) % 2 == 0:
                                    nc.scalar.activation(
                                        o[:, oc : oc + uw],
                                        ps[u][:],
                                        IDENT,
                                        bias=b_sb[:, cb : cb + 1],
                                    )
                                else:
                                    nc.vector.tensor_scalar_add(
                                        o[:, oc : oc + uw],
                                        ps[u][:],
                                        b_sb[:, cb : cb + 1],
                                    )
                                oc += uw
                            dst = out[
                                img,
                                cb * 128 : (cb + 1) * 128,
                                col0 : col0 + wid,
                            ]
                            qs[pi % 2].dma_start(dst, o[:])
                        first_wave = False

    nc.compile()
    return nc


_NC_CACHE = None


def _get_nc():
    global _NC_CACHE
    if _NC_CACHE is None:
        _NC_CACHE = _build_nc()
    return _NC_CACHE


def _host_prep(x, W, b, lora_A, lora_B):
    """Layout + bf16 rounding on host (identical RNE rounding to the DVE
    casts the kernel previously performed on device); no other arithmetic."""
    x = np.ascontiguousarray(x, dtype=np.float32)
    xp_all = np.zeros((B, C_IN, HP, WP), dtype=np.float32)
    xp_all[:, :, 1 : H + 1, 1 : W_DIM + 1] = x
    xp_all = xp_all.reshape(B, C_IN, HP * WP).astype(BF16_NP)

    # [co, ci, kh, kw] -> [ci, k, co]
    wt = (
        np.ascontiguousarray(
            np.asarray(W, dtype=np.float32).reshape(C_OUT, C_IN, 9).transpose(1, 2, 0)
        )
        .reshape(C_IN, 9 * C_OUT)
        .astype(BF16_NP)
    )
    # lora_A [r, ci*9+k] -> [r, k, ci]; lora_B [co, r] -> [r, co]; bundled
    at = np.asarray(lora_A, dtype=np.float32).reshape(RANK, C_IN, 9).transpose(0, 2, 1)
    bt = np.asarray(lora_B, dtype=np.float32).T
    ab = np.concatenate(
        [at.reshape(RANK, 9 * C_IN), bt, np.zeros((RANK, C_OUT), np.float32)], axis=1
    ).astype(BF16_NP)
    ab = np.ascontiguousarray(ab)
    # [256] -> [128, 2]: bv[p, cb] = b[cb*128 + p]
    bv = np.ascontiguousarray(np.asarray(b, dtype=np.float32).reshape(2, 128).T)
    return xp_all, wt, ab, bv


def run(x, W, b, lora_A, lora_B, trace=False):
    """Run the kernel on 8 cores; returns (full_output, BassKernelResults)."""
    xp_all, wt, ab, bv = _host_prep(x, W, b, lora_A, lora_B)
    nc = _get_nc()
    in_maps = []
    for c in range(N_CORES):
        in_maps.append(
            {
                "xp": np.ascontiguousarray(xp_all[c * B_LOC : (c + 1) * B_LOC]),
                "wt": wt,
                "ab": ab,
                "bv": bv,
            }
        )
    res = run_bass_kernel_spmd(
        nc, in_maps, core_ids=list(range(N_CORES)), trace=trace
    )
    out = np.concatenate(
        [r["out"].astype(np.float32) for r in res.results], axis=0
    )
    return out.reshape(B, C_OUT, H, W_DIM), res


def kernel(x, W, b, lora_A, lora_B):
    out, _ = run(x, W, b, lora_A, lora_B, trace=False)
    return out


# revision 36
# speedup vs baseline: 1.0036x; 1.0036x over previous
"""Conv2d(128->256, 3x3, pad 1) with LoRA (rank 8) — Trainium2 Bass kernel.

Strategy:
  - Data-parallel over batch: 16 images -> 2 per core x 8 cores. Conv weights
    and LoRA A/B replicated.
  - LoRA folds into the conv weight on device (conv is linear in weights):
        W_eff = W + (alpha/rank) * (B @ A).reshape(C_OUT, C_IN, 3, 3)
    via 9 tiny PE matmuls (K=8) + fused DVE scalar_tensor_tensor adds.
  - The 3x3 conv itself = 9 shifted matmuls accumulating in PSUM:
        out[co, pix] += W_eff[co, :, kh, kw]^T @ x_shift[ci, pix]
    with K = C_IN = 128 (partition dim), M = 128 (co block), N = 512
    (8 image rows x 64 cols) in bf16 — 1 col/cycle = full PE rate; the
    288-matmul stream is the bf16 roofline (~62 us warm).
  - All tensor I/O is bf16 (host does the identical RNE rounding the DVE
    used to do; output is written bf16 and upcast on host). Halves DMA
    traffic, removes every DVE cast.
  - Head: three f32 warm-up matmuls release the PE clock gate (HAM) before
    the conv; A/B land first (one bundled DMA) -> LoRA matmuls; wt arrives
    in five fold-order pieces spread over all three DMA queues (each HW
    queue only sustains ~55-90 GB/s early and completion sems lag the data
    by ~1.5-2.5us); the conv is emitted in k-minor 4-row-group waves so
    the in-order PE queue chases the weff folds without stalling.
  - x image 1 + bias + part of wt stream via the gpsimd SWDGE queue,
    keeping both HWDGE queues free for x0/wt in and output tiles out.
  - Output tiles drain as row-group pairs (one DMA per pair); the final
    wave runs k-major with per-tile DMAs so only 128KB drains after the
    last matmul. The end-of-kernel semaphore-reset parade (~9us, fixed
    framework epilogue) plus ~6us of engine bring-up are measurement
    constants this kernel cannot remove.
"""

import numpy as np
import ml_dtypes

import concourse.bass as bass
import concourse.tile as tile
from concourse.tile import add_dep_helper
from concourse import bacc, mybir
from concourse.bass_utils import run_bass_kernel_spmd

N_CORES = 8
B, C_IN, H, W_DIM = 16, 128, 64, 64
C_OUT = 256
RANK = 8
SCALING = 2.0  # alpha/rank = 16/8
HP, WP = H + 2, W_DIM + 2  # zero-padded image dims
B_LOC = B // N_CORES  # images per core
NPIX = H * W_DIM  # 4096
ROWS_PER_TILE = 8  # output rows per matmul group -> N = 8*64 = 512
N_RG = H // ROWS_PER_TILE  # 8 row groups

F32 = mybir.dt.float32
BF16 = mybir.dt.bfloat16
IDENT = mybir.ActivationFunctionType.Identity
BF16_NP = ml_dtypes.bfloat16


def _build_nc():
    nc = bacc.Bacc(
        "TRN2",
        target_bir_lowering=False,
        debug=False,
        num_devices=N_CORES,
    )

    xp = nc.dram_tensor("xp", [B_LOC, C_IN, HP * WP], BF16, kind="ExternalInput").ap()
    wt = nc.dram_tensor("wt", [C_IN, 9 * C_OUT], BF16, kind="ExternalInput").ap()
    # at, bt and 256 zero columns bundled: [8, 9*128 | 256 | 256] -> one DMA.
    # The zero tail lets warm-filler matmuls use a 512-wide moving operand.
    ab = nc.dram_tensor(
        "ab", [RANK, 9 * C_IN + 2 * C_OUT], BF16, kind="ExternalInput"
    ).ap()
    bv = nc.dram_tensor("bv", [128, 2], F32, kind="ExternalInput").ap()
    out = nc.dram_tensor("out", [B_LOC, C_OUT, NPIX], BF16, kind="ExternalOutput").ap()

    with tile.TileContext(nc) as tc:
        with (
            tc.tile_pool(name="persist", bufs=1) as persist,
            tc.tile_pool(name="outp", bufs=6) as outp,
            tc.tile_pool(name="psum", bufs=8, space="PSUM") as psum,
        ):
            # --- persistent SBUF tiles (all bf16 straight off DMA) ----------
            x_sb = [
                persist.tile([C_IN, HP * WP], BF16, name=f"x_sb{i}")
                for i in range(B_LOC)
            ]
            wt_sb = persist.tile([C_IN, 9 * C_OUT], BF16, name="wt_sb")
            weff = persist.tile([C_IN, 9 * C_OUT], BF16, name="weff")
            ab_sb = persist.tile([RANK, 9 * C_IN + 2 * C_OUT], BF16, name="ab_sb")
            b_sb = persist.tile([128, 2], F32, name="b_sb")
            warm_sb = persist.tile([128, 512], F32, name="warm_sb")

            # --- input DMAs ------------------------------------------------
            # Queue FIFO order = priority order; each DMA_DIRECT2D costs
            # ~0.65us of issue time on its queue engine and completion sems
            # lag the data by ~1.5-2us (HBM write receipt). Critical path to
            # the first conv matmul: ab -> LoRA MMs -> (with wt q0) weff
            # fold 0; x0 rows chase the first wave's row-groups.
            qs = [nc.sync, nc.scalar]
            # Measured: each queue sustains only ~65-90 GB/s early and
            # completion sems lag data by ~1.5-2.5us, so the critical DMAs
            # sit at the FRONT of their queues and wt arrives in five
            # 512-col pieces (one per weff fold, in fold order) spread over
            # all three queues so the folds pace ~evenly.
            #   sync:   ab (gates LoRA), wt p0, wt p1
            #   scalar: x0 wave A rows (gates conv rg0-3), wt p2, x0 wave B
            #   gpsimd (SWDGE): wt p3, wt p4, bias, x1
            # p0 ahead of ab on sync: fold A (gated by p0's completion sem)
            # is on the conv critical path, while the LoRA matmuls (gated
            # by ab) are PE-queue-bound behind the f32 warm block anyway.
            nc.sync.dma_start(wt_sb[:, 0:512], wt[:, 0:512])
            nc.sync.dma_start(ab_sb[:], ab)
            xa1 = 18 * WP  # rows 0..17: conv rg0-1 (+ rg2's upper rows)
            xa = 34 * WP  # rows 0..33 cover conv wave A (rg0-3)
            nc.scalar.dma_start(x_sb[0][:, :xa1], xp[0, :, :xa1])
            nc.scalar.dma_start(x_sb[0][:, xa1:xa], xp[0, :, xa1:xa])
            nc.scalar.dma_start(wt_sb[:, 1024:1536], wt[:, 1024:1536])
            nc.scalar.dma_start(x_sb[0][:, xa:], xp[0, :, xa:])
            # The warm tile memset runs on gpsimd ahead of its DMA issues:
            # gpsimd's queue is ready ~1.3us before the DVE's, so the f32
            # warm-up matmuls (gated only by this memset) start earlier.
            nc.gpsimd.memset(warm_sb[:], 0.0)
            # wt p1 rides the otherwise-idle SWDGE queue: on sync (3rd slot
            # behind ab+p0 at ~55 GB/s early) its completion lagged to
            # ~+9us and stalled the fold chain at fold B.
            nc.gpsimd.dma_start(wt_sb[:, 512:1024], wt[:, 512:1024])
            nc.gpsimd.dma_start(wt_sb[:, 1536:2048], wt[:, 1536:2048])
            nc.gpsimd.dma_start(wt_sb[:, 2048:], wt[:, 2048:])
            nc.gpsimd.dma_start(b_sb[:], bv)
            nc.gpsimd.dma_start(x_sb[1][:], xp[1, :, :])

            # --- PE warm-up ------------------------------------------------
            # The HAM clock gate holds the PE at 1.2 GHz until ~3.4us of
            # sustained busy. Three f32 N=512 matmuls (4 cycles/row, dense
            # array activity, no DMA deps) reliably release it — bf16
            # filler streams and 2x f32 measurably do NOT.
            lps = [
                psum.tile([128, 512], F32, tag="lps", bufs=3, name=f"lps{j}")
                for j in range(5)
            ]
            for _ in range(3):
                nc.tensor.matmul(
                    lps[0][:], warm_sb[:, :128], warm_sb[:], start=True, stop=True
                )

            # --- fold LoRA into the conv weight ----------------------------
            # lps[j][:, (k%2)*256:...] = (A_k)^T @ B^T  for k = 2j, 2j+1
            # weff[:, k*256+co] = wt[:, k*256+co] + 2 * lps[...]
            # Tile dependency tracking is backward-looking: a fold emitted
            # before its lps writer silently reads a stale bank. So k0..k5
            # and folds A..C are emitted here; k6..k8 (which must wait for
            # fold A to free an lps bank anyway) AND folds D/E are emitted
            # together inside conv wave A after its k1 row — the conv k0
            # row then starts right at fold-A completion and k6..k8 fill
            # the PE while the conv waits for fold B.
            def lora_mm(k):
                nc.tensor.matmul(
                    lps[k // 2][:, (k % 2) * 256 : (k % 2) * 256 + 256],
                    ab_sb[:, k * 128 : (k + 1) * 128],
                    ab_sb[:, 9 * C_IN : 9 * C_IN + 256],
                    start=True,
                    stop=True,
                )

            for k in range(6):
                lora_mm(k)


            # DVE stream is FIFO and the scheduler's DMA-latency model is
            # optimistic: chain the weff folds in k order so conv wave 0
            # starts progressively off fold #0.
            def chain(inst, prev, why):
                if prev is not None:
                    add_dep_helper(inst.ins, prev.ins, sync=False, reason=why)
                return inst

            def fold(j, link):
                w = 512 if j < 4 else 256
                return chain(
                    nc.vector.scalar_tensor_tensor(
                        weff[:, j * 512 : j * 512 + w],
                        lps[j][:, :w],
                        SCALING,
                        wt_sb[:, j * 512 : j * 512 + w],
                        op0=mybir.AluOpType.mult,
                        op1=mybir.AluOpType.add,
                    ),
                    link,
                    "weff fold k order",
                )

            link = None
            for j in range(3):
                link = fold(j, link)

            # --- the conv: 9 accumulating shift-matmuls per output tile ----
            # Emitted k-minor in 4-row-group waves: the in-order PE queue
            # then needs weff fold j only ~8 matmuls after fold j-1, so it
            # chases the DVE chain without stalling, and each weight is
            # loaded once per wave instead of once per tile.
            first_wave = True
            for img in range(B_LOC):
                x_r = x_sb[img][:].rearrange("p (h w) -> p h w", w=WP)
                for cb in range(2):
                    for wv in range(2):
                        rgs = [wv * 4 + j for j in range(4)]
                        last_wave = img == B_LOC - 1 and cb == 1 and wv == 1
                        # units: (rg, row offset within rg, n rows). The
                        # LAST row group is split into two 4-row half
                        # tiles in separate PSUM banks: their bias-adds
                        # then run on ACT and DVE in PARALLEL and two
                        # 64KB DMAs drain on both queues, shortening the
                        # final post-matmul drain.
                        if not last_wave:
                            units = [(rg, 0, 8) for rg in rgs]
                        else:
                            units = [(4, 0, 8), (5, 0, 8), (6, 0, 8),
                                     (7, 0, 4), (7, 4, 4)]
                        ps = {
                            u: psum.tile(
                                [128, u[2] * 64], F32, tag="ps", bufs=5,
                                name=f"ps{img}_{cb}_{u[0]}_{u[1]}",
                            )
                            for u in units
                        }
                        # k-minor: the PE chases the weff folds (needed for
                        # the first waves). The LAST wave is k-major so its
                        # tiles COMPLETE staggered and their drains overlap
                        # the matmuls instead of piling up at the end.
                        kloop = (
                            [(k, u) for k in range(9) for u in units]
                            if not last_wave
                            else [(k, u) for u in units for k in range(9)]
                        )
                        for k, u in kloop:
                            rg, roff, nrows = u
                            dh, dw = k // 3 - 1, k % 3 - 1
                            lhsT = weff[
                                :, k * 256 + cb * 128 : k * 256 + cb * 128 + 128
                            ]
                            h0 = rg * ROWS_PER_TILE + roff
                            rhs = x_r[
                                :,
                                h0 + 1 + dh : h0 + 1 + dh + nrows,
                                1 + dw : 65 + dw,
                            ]
                            nc.tensor.matmul(
                                ps[u][:],
                                lhsT,
                                rhs,
                                start=(k == 0),
                                stop=(k == 8),
                            )
                            if first_wave and k == 1 and u == units[-1]:
                                # LoRA k6..k8 + weff folds D/E, emitted
                                # HERE so the folds follow their writers
                                # in program order (correctness) while the
                                # matmuls fill the fold-B chase window.
                                for kk in range(6, 9):
                                    lora_mm(kk)
                                link2 = fold(3, link)
                                fold(4, link2)

                        # drain: PSUM -> bf16 SBUF (+bias) per unit,
                        # alternating ACT/DVE; one out-DMA per rg PAIR
                        # (fewer DMA instructions -> less issue time on the
                        # queues). The FINAL wave uses one DMA per unit on
                        # alternating queues so only the last 64KB half
                        # tile drains after the final matmul.
                        if not last_wave:
                            prs = [(units[0], units[1]), (units[2], units[3])]
                        else:
                            prs = [(u,) for u in units]
                        for pi, pair in enumerate(prs):
                            wid = sum(64 * u[2] for u in pair)
                            o = outp.tile(
                                [128, wid], BF16, tag="o",
                                name=f"o{img}_{cb}_{pair[0][0]}_{pair[0][1]}",
                            )
                            col0 = pair[0][0] * 512 + pair[0][1] * 64
                            oc = 0
                            for h, u in enumerate(pair):
                                uw = 64 * u[2]
                                ti = (img * 2 + cb) * N_RG + u[0] + u[1] // 4
                                if (ti + (h if not last_wave else 0)) % 2 == 0:
                                    nc.scalar.activation(
                                        o[:, oc : oc + uw],
                                        ps[u][:],
                                        IDENT,
                                        bias=b_sb[:, cb : cb + 1],
                                    )
                                else:
                                    nc.vector.tensor_scalar_add(
                                        o[:, oc : oc + uw],
                                        ps[u][:],
                                        b_sb[:, cb : cb + 1],
                                    )
                                oc += uw
                            dst = out[
                                img,
                                cb * 128 : (cb + 1) * 128,
                                col0 : col0 + wid,
                            ]
                            qs[pi % 2].dma_start(dst, o[:])
                        first_wave = False

    nc.compile()
    return nc


_NC_CACHE = None


def _get_nc():
    global _NC_CACHE
    if _NC_CACHE is None:
        _NC_CACHE = _build_nc()
    return _NC_CACHE


def _host_prep(x, W, b, lora_A, lora_B):
    """Layout + bf16 rounding on host (identical RNE rounding to the DVE
    casts the kernel previously performed on device); no other arithmetic."""
    x = np.ascontiguousarray(x, dtype=np.float32)
    xp_all = np.zeros((B, C_IN, HP, WP), dtype=np.float32)
    xp_all[:, :, 1 : H + 1, 1 : W_DIM + 1] = x
    xp_all = xp_all.reshape(B, C_IN, HP * WP).astype(BF16_NP)

    # [co, ci, kh, kw] -> [ci, k, co]
    wt = (
        np.ascontiguousarray(
            np.asarray(W, dtype=np.float32).reshape(C_OUT, C_IN, 9).transpose(1, 2, 0)
        )
        .reshape(C_IN, 9 * C_OUT)
        .astype(BF16_NP)
    )
    # lora_A [r, ci*9+k] -> [r, k, ci]; lora_B [co, r] -> [r, co]; bundled
    at = np.asarray(lora_A, dtype=np.float32).reshape(RANK, C_IN, 9).transpose(0, 2, 1)
    bt = np.asarray(lora_B, dtype=np.float32).T
    ab = np.concatenate(
        [at.reshape(RANK, 9 * C_IN), bt, np.zeros((RANK, C_OUT), np.float32)], axis=1
    ).astype(BF16_NP)
    ab = np.ascontiguousarray(ab)
    # [256] -> [128, 2]: bv[p, cb] = b[cb*128 + p]
    bv = np.ascontiguousarray(np.asarray(b, dtype=np.float32).reshape(2, 128).T)
    return xp_all, wt, ab, bv


def run(x, W, b, lora_A, lora_B, trace=False):
    """Run the kernel on 8 cores; returns (full_output, BassKernelResults)."""
    xp_all, wt, ab, bv = _host_prep(x, W, b, lora_A, lora_B)
    nc = _get_nc()
    in_maps = []
    for c in range(N_CORES):
        in_maps.append(
            {
                "xp": np.ascontiguousarray(xp_all[c * B_LOC : (c + 1) * B_LOC]),
                "wt": wt,
                "ab": ab,
                "bv": bv,
            }
        )
    res = run_bass_kernel_spmd(
        nc, in_maps, core_ids=list(range(N_CORES)), trace=trace
    )
    out = np.concatenate(
        [r["out"].astype(np.float32) for r in res.results], axis=0
    )
    return out.reshape(B, C_OUT, H, W_DIM), res


def kernel(x, W, b, lora_A, lora_B):
    out, _ = run(x, W, b, lora_A, lora_B, trace=False)
    return out
